# revision 16
# baseline (speedup 1.0000x reference)
"""Trainium2 Bass kernel for Tacotron2-style location-sensitive attention.

Reference computation (B=64, T=2048, ATTN_RNN=1024, EMBED=512, ATTN=128,
N_FILTERS=KERNEL=31):
    q = x @ Wq.T                               [B, 128]
    loc = conv1d(aw_cat, Wconv, pad=15)        [B, 31, T]
    loc = einsum("bct,dc->btd", loc, Wdense)   [B, T, 128]
    e = tanh(q[:,None,:] + loc + pm)           [B, T, 128]
    s = e @ Ws + b                             [B, T]
    w = softmax(s, axis=1)                     [B, T]
    ctx = einsum("bt,bte->be", w, memory)      [B, 512]
    returns (ctx, w)

Device strategy (8 NeuronCores, data-parallel over batch, 8 batches/core):
  * Host folds conv+dense weights: Wcomb[d, (i,k)] = sum_c Wdense[d,c]*Wconv[c,i,k]
    so loc^T = WcombT.T @ A where A[(i,k), t] = awpad[i, t+k] is built by an
    overlapping-window DMA read from the zero-padded attention weights.
  * q is computed on-device ([d, b] psum layout) and added as the per-partition
    activation bias of the tanh.  processed_memory (pre-transposed to [d, t] on
    host) is added into the loc PSUM with an identity matmul.
  * scores are produced in a (t-partition, t-tile) layout by 16 small matmuls
    per batch with the e-tile as the stationary operand, so exp() is one ACT op
    and the context matmul can consume u=exp(s) columns directly as stationary.
  * b_score is dropped (softmax shift invariant).  Softmax normalization and
    the final division happen on host: w = u/Z, ctx = (u @ memory)/Z, Z = sum u.
"""

import os
import sys

sys.path.insert(0, "/opt/trn_rl_repo")

import numpy as np

B, T = 64, 2048
NCORES = 8
BPC = B // NCORES  # batches per core
RNN, E, D = 1024, 512, 128
NF = KW = 31
PAD = (KW - 1) // 2
IK = 2 * KW  # 62 folded (in_channel, tap) rows
NT = T // 128  # 16 t-tiles of 128
NG = 4  # groups of 512 along t
TP = T + 2 * PAD  # 2078

MEM_BF16 = os.environ.get("MEM_BF16", "1") == "1"
PM_BF16 = os.environ.get("PM_BF16", "1") == "1"
A_BF16 = os.environ.get("A_BF16", "1") == "1"
CTX_COLTILE = os.environ.get("CTX_COLTILE", "1") == "1"

_CACHE = {}


def _build(mem_bf16, pm_bf16, a_bf16, coltile=True, repeat=1):
    import concourse.bass as bass
    import concourse.tile as tile
    from concourse import bacc, mybir
    from concourse.masks import make_identity

    f32 = mybir.dt.float32
    bf16 = mybir.dt.bfloat16
    mdt = bf16 if mem_bf16 else f32
    pdt = bf16 if pm_bf16 else f32
    adt = bf16 if a_bf16 else f32

    nc = bacc.Bacc(None, target_bir_lowering=False, name="loc_attn")

    wcombT = nc.dram_tensor("wcombT", [IK, D], adt, kind="ExternalInput")
    wqT = nc.dram_tensor("wqT", [RNN, D], f32, kind="ExternalInput")
    ws = nc.dram_tensor("ws", [D, 1], f32, kind="ExternalInput")
    xT = nc.dram_tensor("xT", [128, RNN // 128, BPC], f32, kind="ExternalInput")
    awpad = nc.dram_tensor("awpad", [BPC, 2, TP], adt, kind="ExternalInput")
    pmT = nc.dram_tensor("pmT", [BPC, D, T], pdt, kind="ExternalInput")
    mem = nc.dram_tensor("mem", [BPC, T, E], mdt, kind="ExternalInput")
    u_out = nc.dram_tensor("u_out", [128, BPC, NT], f32, kind="ExternalOutput")
    ctx_out = nc.dram_tensor("ctx_out", [BPC, E], f32, kind="ExternalOutput")

    KT = RNN // 128  # 8 k-tiles for the query projection

    with tile.TileContext(nc) as tc:
        with (
            tc.tile_pool(name="const", bufs=1) as const_pool,
            tc.tile_pool(name="a", bufs=2) as a_pool,
            tc.tile_pool(name="pm", bufs=2) as pm_pool,
            tc.tile_pool(name="mem", bufs=5 if coltile else 2) as mem_pool,
            tc.tile_pool(name="e", bufs=3) as e_pool,
            tc.tile_pool(name="ps_loc", bufs=3, space="PSUM") as ps_loc_pool,
            tc.tile_pool(name="ps_sc", bufs=2, space="PSUM") as ps_sc_pool,
            tc.tile_pool(name="ps_ctx", bufs=2, space="PSUM") as ps_ctx_pool,
            tc.tile_pool(name="ps_q", bufs=1, space="PSUM") as ps_q_pool,
        ):
            # ---- constants ----
            wcomb_sb = const_pool.tile([IK, D], adt)
            nc.sync.dma_start(out=wcomb_sb, in_=wcombT[:, :])

            ws_sb = const_pool.tile([D, 1], f32)
            nc.sync.dma_start(out=ws_sb, in_=ws[:, :])

            ident = const_pool.tile([128, 128], pdt)
            make_identity(nc, ident)

            wq_sb = const_pool.tile([128, KT, D], f32)
            nc.sync.dma_start(
                out=wq_sb,
                in_=bass.AP(tensor=wqT, offset=0, ap=[[D, 128], [128 * D, KT], [1, D]]),
            )
            xT_sb = const_pool.tile([128, KT, BPC], f32)
            nc.sync.dma_start(out=xT_sb, in_=xT[:, :, :])

            # ---- query projection: q[d, b] ----
            q_ps = ps_q_pool.tile([D, BPC], f32)
            for kt in range(KT):
                nc.tensor.matmul(
                    out=q_ps,
                    lhsT=wq_sb[:, kt, :],
                    rhs=xT_sb[:, kt, :],
                    start=(kt == 0),
                    stop=(kt == KT - 1),
                )
            q_sb = const_pool.tile([D, BPC], f32)
            nc.vector.tensor_copy(q_sb, q_ps)

            u_sb = const_pool.tile([128, BPC, NT], f32)
            u_mm = u_sb
            if mem_bf16:
                u_mm = const_pool.tile([128, BPC, NT], bf16)
            if coltile:
                # ctx rows live on partitions {0,32,64,96}; one column per quad
                ctx_sb = const_pool.tile([128, BPC // 4, E], f32)
            else:
                ctx_sb = const_pool.tile([1, BPC * E], f32)

            def front_half(b):
                """loc conv + pm add + tanh + scores + exp for one batch;
                returns this batch's memory tile."""
                a_t = a_pool.tile([IK, T], adt)
                for i in range(2):
                    nc.sync.dma_start(
                        out=a_t[i * KW : (i + 1) * KW, :],
                        in_=bass.AP(
                            tensor=awpad,
                            offset=(b * 2 + i) * TP,
                            ap=[[1, KW], [1, T]],
                        ),
                    )

                pm_t = pm_pool.tile([D, T], pdt)
                nc.sync.dma_start(out=pm_t, in_=pmT[b])

                mem_t = mem_pool.tile([128, NT, E], mdt)
                for g in range(2):
                    nc.sync.dma_start(
                        out=mem_t[:, 8 * g : 8 * g + 8, :],
                        in_=bass.AP(
                            tensor=mem,
                            offset=b * T * E + g * 1024 * E,
                            ap=[[E, 128], [128 * E, 8], [1, E]],
                        ),
                    )

                sc_ps = ps_sc_pool.tile([128, NT], f32)
                for g in range(NG):
                    loc_ps = ps_loc_pool.tile([D, 512], f32)
                    nc.tensor.matmul(
                        out=loc_ps,
                        lhsT=wcomb_sb,
                        rhs=a_t[:, g * 512 : (g + 1) * 512],
                        start=True,
                        stop=False,
                    )
                    nc.tensor.matmul(
                        out=loc_ps,
                        lhsT=ident,
                        rhs=pm_t[:, g * 512 : (g + 1) * 512],
                        start=False,
                        stop=True,
                    )
                    e_t = e_pool.tile([D, 512], f32)
                    nc.scalar.activation(
                        out=e_t,
                        in_=loc_ps,
                        func=mybir.ActivationFunctionType.Tanh,
                        bias=q_sb[:, b : b + 1],
                        scale=1.0,
                    )
                    for j in range(4):
                        it = g * 4 + j
                        nc.tensor.matmul(
                            out=sc_ps[:, it : it + 1],
                            lhsT=e_t[:, j * 128 : (j + 1) * 128],
                            rhs=ws_sb,
                            start=True,
                            stop=True,
                        )

                nc.scalar.activation(
                    out=u_sb[:, b, :],
                    in_=sc_ps,
                    func=mybir.ActivationFunctionType.Exp,
                )
                if mem_bf16:
                    nc.vector.tensor_copy(u_mm[:, b, :], u_sb[:, b, :])
                return mem_t

            for _rep in range(repeat):
                if coltile:
                    for qd in range(BPC // 4):
                        mem_ts = [front_half(4 * qd + j) for j in range(4)]
                        ctx_ps = ps_ctx_pool.tile([128, E], f32)
                        for it in range(NT):
                            for j in range(4):
                                nc.tensor.matmul(
                                    out=ctx_ps[32 * j : 32 * j + 1, :],
                                    lhsT=u_mm[:, 4 * qd + j, it : it + 1],
                                    rhs=mem_ts[j][:, it, :],
                                    start=(it == 0),
                                    stop=(it == NT - 1),
                                    tile_position=(0, 32 * j),
                                )
                        for j in range(4):
                            nc.vector.tensor_copy(
                                ctx_sb[32 * j : 32 * j + 1, qd, :],
                                ctx_ps[32 * j : 32 * j + 1, :],
                            )
                else:
                    for b in range(BPC):
                        mem_t = front_half(b)
                        ctx_ps = ps_ctx_pool.tile([1, E], f32)
                        for it in range(NT):
                            nc.tensor.matmul(
                                out=ctx_ps,
                                lhsT=u_mm[:, b, it : it + 1],
                                rhs=mem_t[:, it, :],
                                start=(it == 0),
                                stop=(it == NT - 1),
                            )
                        nc.vector.tensor_copy(
                            ctx_sb[:, b * E : (b + 1) * E], ctx_ps
                        )

            nc.sync.dma_start(out=u_out[:, :, :], in_=u_sb)
            if coltile:
                for qd in range(BPC // 4):
                    nc.sync.dma_start(
                        out=ctx_out[4 * qd : 4 * qd + 4, :],
                        in_=ctx_sb[0:128:32, qd, :],
                    )
            else:
                nc.sync.dma_start(
                    out=bass.AP(
                        tensor=ctx_out, offset=0, ap=[[BPC * E, 1], [1, BPC * E]]
                    ),
                    in_=ctx_sb,
                )

    nc.finalize()
    return nc


def _get_nc(repeat=1):
    key = (MEM_BF16, PM_BF16, A_BF16, CTX_COLTILE, repeat)
    if key not in _CACHE:
        _CACHE[key] = _build(MEM_BF16, PM_BF16, A_BF16, CTX_COLTILE, repeat)
    return _CACHE[key]


def _prep_inputs(x, memory, pm, aw_cat, W_query, W_conv, W_dense, W_score):
    """Host-side weight folding + per-core layout prep (layout only / tiny)."""
    import ml_dtypes

    bf16 = ml_dtypes.bfloat16
    f32 = np.float32

    # Wcomb[(i,k), d] = sum_c Wdense[d, c] * Wconv[c, i, k]
    wcombT = np.einsum("dc,cik->ikd", W_dense, W_conv).reshape(IK, D)
    wqT = np.ascontiguousarray(W_query.T).astype(f32)  # [1024, 128]
    ws = np.ascontiguousarray(W_score.reshape(1, D).T).astype(f32)  # [128, 1]

    wcombT = wcombT.astype(bf16 if A_BF16 else f32)
    mdt = bf16 if MEM_BF16 else f32
    pdt = bf16 if PM_BF16 else f32
    adt = bf16 if A_BF16 else f32

    in_maps = []
    for c in range(NCORES):
        sl = slice(c * BPC, (c + 1) * BPC)
        x_c = np.asarray(x[sl], dtype=f32)  # [8, 1024]
        xT = np.ascontiguousarray(
            x_c.reshape(BPC, RNN // 128, 128).transpose(2, 1, 0)
        )  # [128, kt, b]
        awpad = np.zeros((BPC, 2, TP), dtype=adt)
        awpad[:, :, PAD : PAD + T] = np.asarray(aw_cat[sl], dtype=f32)
        pmT = np.ascontiguousarray(
            np.asarray(pm[sl]).transpose(0, 2, 1)
        ).astype(pdt)  # [8, 128, 2048]
        mem_c = np.ascontiguousarray(np.asarray(memory[sl])).astype(mdt)
        in_maps.append(
            {
                "wcombT": wcombT,
                "wqT": wqT,
                "ws": ws,
                "xT": xT,
                "awpad": awpad,
                "pmT": pmT,
                "mem": mem_c,
            }
        )
    return in_maps


LAST_EXEC_NS = None


def _bench_nc(nc, in_maps, n_iters=50):
    """Steady-state per-NEFF-execution wall time with device-resident inputs.

    Mirrors bass2jax.run_bass_via_pjrt's multi-core shard_map path, but
    without donation, inputs pre-placed on device, timed over a pipelined
    loop of executions.  Returns seconds per iteration.
    """
    import time

    import jax
    from jax.experimental.shard_map import shard_map
    from jax.sharding import Mesh, NamedSharding, PartitionSpec

    from concourse import mybir
    from concourse.bass2jax import (
        _bass_exec_p,
        install_neuronx_cc_hook,
        partition_id_tensor,
    )

    install_neuronx_cc_hook()

    partition_name = nc.partition_id_tensor.name if nc.partition_id_tensor else None
    in_names, out_names, out_avals, zero_outs = [], [], [], []
    for alloc in nc.m.functions[0].allocations:
        if not isinstance(alloc, mybir.MemoryLocationSet):
            continue
        name = alloc.memorylocations[0].name
        if alloc.kind == "ExternalInput":
            if name != partition_name:
                in_names.append(name)
        elif alloc.kind == "ExternalOutput":
            out_names.append(name)
            shape = tuple(alloc.tensor_shape)
            dtype = mybir.dt.np(alloc.dtype)
            out_avals.append(jax.core.ShapedArray(shape, dtype))
            zero_outs.append(np.zeros(shape, dtype))
    n_params = len(in_names)
    all_in_names = in_names + out_names
    if partition_name is not None:
        all_in_names_final = all_in_names + [partition_name]
    else:
        all_in_names_final = all_in_names

    def _body(*args):
        operands = list(args)
        if partition_name is not None:
            operands.append(partition_id_tensor())
        outs = _bass_exec_p.bind(
            *operands,
            out_avals=tuple(out_avals),
            in_names=tuple(all_in_names_final),
            out_names=tuple(out_names),
            lowering_input_output_aliases=(),
            sim_require_finite=True,
            sim_require_nnan=True,
            nc=nc,
        )
        return tuple(outs)

    devices = jax.devices()[:NCORES]
    mesh = Mesh(np.asarray(devices), ("core",))
    nspec = n_params + len(out_names)
    f = jax.jit(
        shard_map(
            _body,
            mesh=mesh,
            in_specs=(PartitionSpec("core"),) * nspec,
            out_specs=(PartitionSpec("core"),) * len(out_names),
            check_rep=False,
        ),
        keep_unused=True,
    )
    sharding = NamedSharding(mesh, PartitionSpec("core"))
    concat_in = [
        np.concatenate([np.asarray(in_maps[c][nm]) for c in range(NCORES)], axis=0)
        for nm in in_names
    ] + [np.zeros((NCORES * z.shape[0], *z.shape[1:]), z.dtype) for z in zero_outs]
    dev_args = [jax.device_put(a, sharding) for a in concat_in]

    out = f(*dev_args)
    jax.block_until_ready(out)
    best = None
    for _rep in range(3):
        t0 = time.perf_counter()
        outs = None
        for _ in range(n_iters):
            outs = f(*dev_args)
        jax.block_until_ready(outs)
        dt = (time.perf_counter() - t0) / n_iters
        best = dt if best is None else min(best, dt)
    return best


def _in_maps_from(inputs):
    return _prep_inputs(
        inputs["attention_rnn_last_output"],
        inputs["memory"],
        inputs["processed_memory"],
        inputs["attention_weights_cat"],
        inputs["W_query"],
        inputs["W_conv"],
        inputs["W_dense"],
        inputs["W_score"],
    )


def bench(inputs, n_iters=50):
    return _bench_nc(_get_nc(), _in_maps_from(inputs), n_iters)


def bench_hw(inputs, n_iters=30, repeat=9):
    """Estimate true per-kernel HW time via the difference quotient between a
    NEFF containing `repeat` chained kernel bodies and the single-body NEFF,
    cancelling the per-dispatch axon overhead."""
    in_maps = _in_maps_from(inputs)
    t1 = _bench_nc(_get_nc(), in_maps, n_iters)
    tR = _bench_nc(_get_nc(repeat=repeat), in_maps, n_iters)
    return (tR - t1) / (repeat - 1), t1, tR


def kernel(
    attention_rnn_last_output,
    memory,
    processed_memory,
    attention_weights_cat,
    mask_seq,
    W_query,
    W_conv,
    W_dense,
    W_score,
    b_score,
):
    global LAST_EXEC_NS
    from concourse.bass_utils import run_bass_kernel_spmd

    nc = _get_nc()
    in_maps = _prep_inputs(
        attention_rnn_last_output,
        memory,
        processed_memory,
        attention_weights_cat,
        W_query,
        W_conv,
        W_dense,
        W_score,
    )

    res = run_bass_kernel_spmd(nc, in_maps, core_ids=list(range(NCORES)), trace=False)
    LAST_EXEC_NS = res.exec_time_ns

    mask = np.asarray(mask_seq)
    context = np.empty((B, E), dtype=np.float32)
    w = np.empty((B, T), dtype=np.float32)
    for c in range(NCORES):
        sl = slice(c * BPC, (c + 1) * BPC)
        u = (
            np.asarray(res.results[c]["u_out"], dtype=np.float32)
            .transpose(1, 2, 0)
            .reshape(BPC, T)
        )  # u[b, t] = exp(scores)
        ctx_u = np.asarray(res.results[c]["ctx_out"], dtype=np.float32)
        m_c = mask[sl]
        if m_c.any():
            # masked positions get weight exp(-inf) = 0; context must then be
            # recomputed from the masked weights (slow fallback; spec mask is
            # all-False so this path is never hit in practice)
            u = np.where(m_c, 0.0, u)
            Z = u.sum(axis=1, dtype=np.float64)
            w_c = (u / Z[:, None]).astype(np.float32)
            context[sl] = np.einsum(
                "bt,bte->be", w_c, np.asarray(memory[sl], dtype=np.float32)
            )
            w[sl] = w_c
        else:
            Z = u.sum(axis=1, dtype=np.float64)
            w[sl] = u / Z[:, None]
            context[sl] = ctx_u / Z[:, None]
    return context, w


# revision 17
# speedup vs baseline: 2.0919x; 2.0919x over previous
"""Trainium2 Bass kernel for Tacotron2-style location-sensitive attention.

Reference computation (B=64, T=2048, ATTN_RNN=1024, EMBED=512, ATTN=128,
N_FILTERS=KERNEL=31):
    q = x @ Wq.T                               [B, 128]
    loc = conv1d(aw_cat, Wconv, pad=15)        [B, 31, T]
    loc = einsum("bct,dc->btd", loc, Wdense)   [B, T, 128]
    e = tanh(q[:,None,:] + loc + pm)           [B, T, 128]
    s = e @ Ws + b                             [B, T]
    w = softmax(s, axis=1)                     [B, T]
    ctx = einsum("bt,bte->be", w, memory)      [B, 512]
    returns (ctx, w)

Device strategy (8 NeuronCores, data-parallel over batch, 8 batches/core):
  * Host folds conv+dense weights: Wcomb[d, (i,k)] = sum_c Wdense[d,c]*Wconv[c,i,k]
    so loc^T = WcombT.T @ A where A[(i,k), t] = awpad[i, t+k] is built by an
    overlapping-window DMA read from the zero-padded attention weights.
  * q is computed on-device ([d, b] psum layout) and added as the per-partition
    activation bias of the tanh.  processed_memory (pre-transposed to [d, t] on
    host) is added into the loc PSUM with an identity matmul.
  * scores are produced in a (t-partition, t-tile) layout by 16 small matmuls
    per batch with the e-tile as the stationary operand, so exp() is one ACT op
    and the context matmul can consume u=exp(s) columns directly as stationary.
  * b_score is dropped (softmax shift invariant).  Softmax normalization and
    the final division happen on host: w = u/Z, ctx = (u @ memory)/Z, Z = sum u.
"""

import os
import sys

sys.path.insert(0, "/opt/trn_rl_repo")

import numpy as np

B, T = 64, 2048
NCORES = 8
BPC = B // NCORES  # batches per core
RNN, E, D = 1024, 512, 128
NF = KW = 31
PAD = (KW - 1) // 2
IK = 2 * KW  # 62 folded (in_channel, tap) rows
NT = T // 128  # 16 t-tiles of 128
NG = 4  # groups of 512 along t
TP = T + 2 * PAD  # 2078

MEM_BF16 = os.environ.get("MEM_BF16", "1") == "1"
PM_BF16 = os.environ.get("PM_BF16", "1") == "1"
A_BF16 = os.environ.get("A_BF16", "1") == "1"
CTX_COLTILE = os.environ.get("CTX_COLTILE", "0") == "1"

_CACHE = {}


def _build(mem_bf16, pm_bf16, a_bf16, coltile=True, repeat=1):
    import concourse.bass as bass
    import concourse.tile as tile
    from concourse import bacc, mybir
    from concourse.masks import make_identity

    f32 = mybir.dt.float32
    bf16 = mybir.dt.bfloat16
    mdt = bf16 if mem_bf16 else f32
    pdt = bf16 if pm_bf16 else f32
    adt = bf16 if a_bf16 else f32

    nc = bacc.Bacc(None, target_bir_lowering=False, name="loc_attn")

    wcombT = nc.dram_tensor("wcombT", [IK, D], adt, kind="ExternalInput")
    wqT = nc.dram_tensor("wqT", [RNN, D], f32, kind="ExternalInput")
    ws = nc.dram_tensor("ws", [D, 1], f32, kind="ExternalInput")
    xT = nc.dram_tensor("xT", [128, RNN // 128, BPC], f32, kind="ExternalInput")
    awpad = nc.dram_tensor("awpad", [BPC, 2, TP], adt, kind="ExternalInput")
    pmT = nc.dram_tensor("pmT", [BPC, D, T], pdt, kind="ExternalInput")
    mem = nc.dram_tensor("mem", [BPC, T, E], mdt, kind="ExternalInput")
    u_out = nc.dram_tensor("u_out", [128, BPC, NT], f32, kind="ExternalOutput")
    ctx_out = nc.dram_tensor("ctx_out", [BPC, E], f32, kind="ExternalOutput")

    KT = RNN // 128  # 8 k-tiles for the query projection

    with tile.TileContext(nc) as tc:
        with (
            tc.tile_pool(name="const", bufs=1) as const_pool,
            tc.tile_pool(name="a", bufs=2) as a_pool,
            tc.tile_pool(name="pm", bufs=2) as pm_pool,
            tc.tile_pool(name="mem", bufs=5 if coltile else 2) as mem_pool,
            tc.tile_pool(name="e", bufs=3) as e_pool,
            tc.tile_pool(name="ps_loc", bufs=3, space="PSUM") as ps_loc_pool,
            tc.tile_pool(name="ps_sc", bufs=2, space="PSUM") as ps_sc_pool,
            tc.tile_pool(name="ps_ctx", bufs=2, space="PSUM") as ps_ctx_pool,
            tc.tile_pool(name="ps_q", bufs=1, space="PSUM") as ps_q_pool,
        ):
            # ---- constants ----
            wcomb_sb = const_pool.tile([IK, D], adt)
            nc.sync.dma_start(out=wcomb_sb, in_=wcombT[:, :])

            ws_sb = const_pool.tile([D, 1], f32)
            nc.sync.dma_start(out=ws_sb, in_=ws[:, :])

            ident = const_pool.tile([128, 128], pdt)
            make_identity(nc, ident)

            wq_sb = const_pool.tile([128, KT, D], f32)
            nc.sync.dma_start(
                out=wq_sb,
                in_=bass.AP(tensor=wqT, offset=0, ap=[[D, 128], [128 * D, KT], [1, D]]),
            )
            xT_sb = const_pool.tile([128, KT, BPC], f32)
            nc.sync.dma_start(out=xT_sb, in_=xT[:, :, :])

            # ---- query projection: q[d, b] ----
            q_ps = ps_q_pool.tile([D, BPC], f32)
            for kt in range(KT):
                nc.tensor.matmul(
                    out=q_ps,
                    lhsT=wq_sb[:, kt, :],
                    rhs=xT_sb[:, kt, :],
                    start=(kt == 0),
                    stop=(kt == KT - 1),
                )
            q_sb = const_pool.tile([D, BPC], f32)
            nc.vector.tensor_copy(q_sb, q_ps)

            u_sb = const_pool.tile([128, BPC, NT], f32)
            u_mm = u_sb
            if mem_bf16:
                u_mm = const_pool.tile([128, BPC, NT], bf16)
            if coltile:
                # ctx rows live on partitions {0,32,64,96}; one column per quad
                ctx_sb = const_pool.tile([128, BPC // 4, E], f32)
            else:
                ctx_sb = const_pool.tile([1, BPC * E], f32)

            def front_half(b):
                """loc conv + pm add + tanh + scores + exp for one batch;
                returns this batch's memory tile."""
                a_t = a_pool.tile([IK, T], adt)
                for i in range(2):
                    nc.sync.dma_start(
                        out=a_t[i * KW : (i + 1) * KW, :],
                        in_=bass.AP(
                            tensor=awpad,
                            offset=(b * 2 + i) * TP,
                            ap=[[1, KW], [1, T]],
                        ),
                    )

                pm_t = pm_pool.tile([D, T], pdt)
                nc.sync.dma_start(out=pm_t, in_=pmT[b])

                mem_t = mem_pool.tile([128, NT, E], mdt)
                for g in range(2):
                    nc.sync.dma_start(
                        out=mem_t[:, 8 * g : 8 * g + 8, :],
                        in_=bass.AP(
                            tensor=mem,
                            offset=b * T * E + g * 1024 * E,
                            ap=[[E, 128], [128 * E, 8], [1, E]],
                        ),
                    )

                sc_ps = ps_sc_pool.tile([128, NT], f32)
                for g in range(NG):
                    loc_ps = ps_loc_pool.tile([D, 512], f32)
                    nc.tensor.matmul(
                        out=loc_ps,
                        lhsT=wcomb_sb,
                        rhs=a_t[:, g * 512 : (g + 1) * 512],
                        start=True,
                        stop=False,
                    )
                    nc.tensor.matmul(
                        out=loc_ps,
                        lhsT=ident,
                        rhs=pm_t[:, g * 512 : (g + 1) * 512],
                        start=False,
                        stop=True,
                    )
                    e_t = e_pool.tile([D, 512], f32)
                    nc.scalar.activation(
                        out=e_t,
                        in_=loc_ps,
                        func=mybir.ActivationFunctionType.Tanh,
                        bias=q_sb[:, b : b + 1],
                        scale=1.0,
                    )
                    for j in range(4):
                        it = g * 4 + j
                        nc.tensor.matmul(
                            out=sc_ps[:, it : it + 1],
                            lhsT=e_t[:, j * 128 : (j + 1) * 128],
                            rhs=ws_sb,
                            start=True,
                            stop=True,
                        )

                nc.scalar.activation(
                    out=u_sb[:, b, :],
                    in_=sc_ps,
                    func=mybir.ActivationFunctionType.Exp,
                )
                if mem_bf16:
                    nc.vector.tensor_copy(u_mm[:, b, :], u_sb[:, b, :])
                return mem_t

            for _rep in range(repeat):
                if coltile:
                    for qd in range(BPC // 4):
                        mem_ts = [front_half(4 * qd + j) for j in range(4)]
                        ctx_ps = ps_ctx_pool.tile([128, E], f32)
                        for it in range(NT):
                            for j in range(4):
                                nc.tensor.matmul(
                                    out=ctx_ps[32 * j : 32 * j + 1, :],
                                    lhsT=u_mm[:, 4 * qd + j, it : it + 1],
                                    rhs=mem_ts[j][:, it, :],
                                    start=(it == 0),
                                    stop=(it == NT - 1),
                                    tile_position=(0, 32 * j),
                                )
                        for j in range(4):
                            nc.vector.tensor_copy(
                                ctx_sb[32 * j : 32 * j + 1, qd, :],
                                ctx_ps[32 * j : 32 * j + 1, :],
                            )
                else:
                    for b in range(BPC):
                        mem_t = front_half(b)
                        ctx_ps = ps_ctx_pool.tile([1, E], f32)
                        for it in range(NT):
                            nc.tensor.matmul(
                                out=ctx_ps,
                                lhsT=u_mm[:, b, it : it + 1],
                                rhs=mem_t[:, it, :],
                                start=(it == 0),
                                stop=(it == NT - 1),
                            )
                        nc.vector.tensor_copy(
                            ctx_sb[:, b * E : (b + 1) * E], ctx_ps
                        )

            nc.sync.dma_start(out=u_out[:, :, :], in_=u_sb)
            if coltile:
                for qd in range(BPC // 4):
                    nc.sync.dma_start(
                        out=ctx_out[4 * qd : 4 * qd + 4, :],
                        in_=ctx_sb[0:128:32, qd, :],
                    )
            else:
                nc.sync.dma_start(
                    out=bass.AP(
                        tensor=ctx_out, offset=0, ap=[[BPC * E, 1], [1, BPC * E]]
                    ),
                    in_=ctx_sb,
                )

    nc.finalize()
    return nc


def _get_nc(repeat=1):
    key = (MEM_BF16, PM_BF16, A_BF16, CTX_COLTILE, repeat)
    if key not in _CACHE:
        _CACHE[key] = _build(MEM_BF16, PM_BF16, A_BF16, CTX_COLTILE, repeat)
    return _CACHE[key]


def _prep_inputs(x, memory, pm, aw_cat, W_query, W_conv, W_dense, W_score):
    """Host-side weight folding + per-core layout prep (layout only / tiny)."""
    import ml_dtypes

    bf16 = ml_dtypes.bfloat16
    f32 = np.float32

    # Wcomb[(i,k), d] = sum_c Wdense[d, c] * Wconv[c, i, k]
    wcombT = np.einsum("dc,cik->ikd", W_dense, W_conv).reshape(IK, D)
    wqT = np.ascontiguousarray(W_query.T).astype(f32)  # [1024, 128]
    ws = np.ascontiguousarray(W_score.reshape(1, D).T).astype(f32)  # [128, 1]

    wcombT = wcombT.astype(bf16 if A_BF16 else f32)
    mdt = bf16 if MEM_BF16 else f32
    pdt = bf16 if PM_BF16 else f32
    adt = bf16 if A_BF16 else f32

    in_maps = []
    for c in range(NCORES):
        sl = slice(c * BPC, (c + 1) * BPC)
        x_c = np.asarray(x[sl], dtype=f32)  # [8, 1024]
        xT = np.ascontiguousarray(
            x_c.reshape(BPC, RNN // 128, 128).transpose(2, 1, 0)
        )  # [128, kt, b]
        awpad = np.zeros((BPC, 2, TP), dtype=adt)
        awpad[:, :, PAD : PAD + T] = np.asarray(aw_cat[sl], dtype=f32)
        pmT = np.ascontiguousarray(
            np.asarray(pm[sl]).transpose(0, 2, 1)
        ).astype(pdt)  # [8, 128, 2048]
        mem_c = np.ascontiguousarray(np.asarray(memory[sl])).astype(mdt)
        in_maps.append(
            {
                "wcombT": wcombT,
                "wqT": wqT,
                "ws": ws,
                "xT": xT,
                "awpad": awpad,
                "pmT": pmT,
                "mem": mem_c,
            }
        )
    return in_maps


LAST_EXEC_NS = None


def _bench_nc(nc, in_maps, n_iters=50):
    """Steady-state per-NEFF-execution wall time with device-resident inputs.

    Mirrors bass2jax.run_bass_via_pjrt's multi-core shard_map path, but
    without donation, inputs pre-placed on device, timed over a pipelined
    loop of executions.  Returns seconds per iteration.
    """
    import time

    import jax
    from jax.experimental.shard_map import shard_map
    from jax.sharding import Mesh, NamedSharding, PartitionSpec

    from concourse import mybir
    from concourse.bass2jax import (
        _bass_exec_p,
        install_neuronx_cc_hook,
        partition_id_tensor,
    )

    install_neuronx_cc_hook()

    partition_name = nc.partition_id_tensor.name if nc.partition_id_tensor else None
    in_names, out_names, out_avals, zero_outs = [], [], [], []
    for alloc in nc.m.functions[0].allocations:
        if not isinstance(alloc, mybir.MemoryLocationSet):
            continue
        name = alloc.memorylocations[0].name
        if alloc.kind == "ExternalInput":
            if name != partition_name:
                in_names.append(name)
        elif alloc.kind == "ExternalOutput":
            out_names.append(name)
            shape = tuple(alloc.tensor_shape)
            dtype = mybir.dt.np(alloc.dtype)
            out_avals.append(jax.core.ShapedArray(shape, dtype))
            zero_outs.append(np.zeros(shape, dtype))
    n_params = len(in_names)
    all_in_names = in_names + out_names
    if partition_name is not None:
        all_in_names_final = all_in_names + [partition_name]
    else:
        all_in_names_final = all_in_names

    def _body(*args):
        operands = list(args)
        if partition_name is not None:
            operands.append(partition_id_tensor())
        outs = _bass_exec_p.bind(
            *operands,
            out_avals=tuple(out_avals),
            in_names=tuple(all_in_names_final),
            out_names=tuple(out_names),
            lowering_input_output_aliases=(),
            sim_require_finite=True,
            sim_require_nnan=True,
            nc=nc,
        )
        return tuple(outs)

    devices = jax.devices()[:NCORES]
    mesh = Mesh(np.asarray(devices), ("core",))
    nspec = n_params + len(out_names)
    f = jax.jit(
        shard_map(
            _body,
            mesh=mesh,
            in_specs=(PartitionSpec("core"),) * nspec,
            out_specs=(PartitionSpec("core"),) * len(out_names),
            check_rep=False,
        ),
        keep_unused=True,
    )
    sharding = NamedSharding(mesh, PartitionSpec("core"))
    concat_in = [
        np.concatenate([np.asarray(in_maps[c][nm]) for c in range(NCORES)], axis=0)
        for nm in in_names
    ] + [np.zeros((NCORES * z.shape[0], *z.shape[1:]), z.dtype) for z in zero_outs]
    dev_args = [jax.device_put(a, sharding) for a in concat_in]

    out = f(*dev_args)
    jax.block_until_ready(out)
    best = None
    for _rep in range(3):
        t0 = time.perf_counter()
        outs = None
        for _ in range(n_iters):
            outs = f(*dev_args)
        jax.block_until_ready(outs)
        dt = (time.perf_counter() - t0) / n_iters
        best = dt if best is None else min(best, dt)
    return best


def _in_maps_from(inputs):
    return _prep_inputs(
        inputs["attention_rnn_last_output"],
        inputs["memory"],
        inputs["processed_memory"],
        inputs["attention_weights_cat"],
        inputs["W_query"],
        inputs["W_conv"],
        inputs["W_dense"],
        inputs["W_score"],
    )


def bench(inputs, n_iters=50):
    return _bench_nc(_get_nc(), _in_maps_from(inputs), n_iters)


def bench_hw(inputs, n_iters=30, repeat=9):
    """Estimate true per-kernel HW time via the difference quotient between a
    NEFF containing `repeat` chained kernel bodies and the single-body NEFF,
    cancelling the per-dispatch axon overhead."""
    in_maps = _in_maps_from(inputs)
    t1 = _bench_nc(_get_nc(), in_maps, n_iters)
    tR = _bench_nc(_get_nc(repeat=repeat), in_maps, n_iters)
    return (tR - t1) / (repeat - 1), t1, tR


def kernel(
    attention_rnn_last_output,
    memory,
    processed_memory,
    attention_weights_cat,
    mask_seq,
    W_query,
    W_conv,
    W_dense,
    W_score,
    b_score,
):
    global LAST_EXEC_NS
    from concourse.bass_utils import run_bass_kernel_spmd

    nc = _get_nc()
    in_maps = _prep_inputs(
        attention_rnn_last_output,
        memory,
        processed_memory,
        attention_weights_cat,
        W_query,
        W_conv,
        W_dense,
        W_score,
    )

    res = run_bass_kernel_spmd(nc, in_maps, core_ids=list(range(NCORES)), trace=False)
    LAST_EXEC_NS = res.exec_time_ns

    mask = np.asarray(mask_seq)
    context = np.empty((B, E), dtype=np.float32)
    w = np.empty((B, T), dtype=np.float32)
    for c in range(NCORES):
        sl = slice(c * BPC, (c + 1) * BPC)
        u = (
            np.asarray(res.results[c]["u_out"], dtype=np.float32)
            .transpose(1, 2, 0)
            .reshape(BPC, T)
        )  # u[b, t] = exp(scores)
        ctx_u = np.asarray(res.results[c]["ctx_out"], dtype=np.float32)
        m_c = mask[sl]
        if m_c.any():
            # masked positions get weight exp(-inf) = 0; context must then be
            # recomputed from the masked weights (slow fallback; spec mask is
            # all-False so this path is never hit in practice)
            u = np.where(m_c, 0.0, u)
            Z = u.sum(axis=1, dtype=np.float64)
            w_c = (u / Z[:, None]).astype(np.float32)
            context[sl] = np.einsum(
                "bt,bte->be", w_c, np.asarray(memory[sl], dtype=np.float32)
            )
            w[sl] = w_c
        else:
            Z = u.sum(axis=1, dtype=np.float64)
            w[sl] = u / Z[:, None]
            context[sl] = ctx_u / Z[:, None]
    return context, w
